# revision 10
# baseline (speedup 1.0000x reference)
"""ArcFace loss kernel for 8 Trainium2 NeuronCores.

Strategy (class-parallel, Partial-FC style):
  - Host pre-normalizes weight rows (w_hat = w/||w||, bf16) and input rows
    (S*x_hat, bf16, transposed); each core's device program is a pure bf16
    GEMM: out[b, c] = <S*x_hat_b, w_hat_c> for its 12500-class shard
    (padded to 12544), streamed class-group by class-group.
  - Input chunks [128d, 128b] are the stationary operand; weight columns
    stream as the moving operand, so PSUM comes out batch-major [128b, Nc].
  - All DRAM tensors use group-major layouts so every DMA is one contiguous
    run per partition (max descriptor size, minimal descriptor count).
  - Output is written bf16 (halves the dominant DMA-write traffic) and
    upcast to f32 on the host during the gather. Class groups shrink toward
    the end of the sweep so the final output DMAs flush right behind the
    last matmuls.
  - The ArcFace margin touches one element per row; the host computes the
    512 margin values phi(cos(b, label_b)) in float64 and scatters them
    into the gathered [B, C] output.
"""

import math
import os
import sys

import numpy as np

for _p in ("/opt/trn_rl_repo",):
    if os.path.isdir(_p) and _p not in sys.path:
        sys.path.insert(0, _p)

import ml_dtypes

S = 30.0
MARGIN = 0.5
COS_M = math.cos(MARGIN)
SIN_M = math.sin(MARGIN)
TH = math.cos(math.pi - MARGIN)
MM = math.sin(math.pi - MARGIN) * MARGIN

B, D, C = 512, 512, 100000
NCORES = 8
CSH = C // NCORES            # 12500 classes per core
CPAD = 12544                 # padded to 98*128
BT = B // 128                # 4 batch tiles
DCH = D // 128               # 4 contraction chunks

# Weight-prefetch groups (columns). Small first groups so compute starts
# early and the sync ring stays ahead of the matmul sweep.
W_GROUPS = [512, 512, 512, 1024, 1024, 2048, 2048, 2048, 2048, 512, 256]
# Output-store groups (columns). Small tail groups so the final stores
# flush immediately behind the last matmuls.
O_GROUPS = [512, 1024, 2048, 2048, 2048, 2048, 1536, 1024, 256]
assert sum(W_GROUPS) == CPAD and sum(O_GROUPS) == CPAD
# PSUM chunk list (c0, ncols): 512-wide, subdividing both group patterns.
CHUNKS = [(c, min(512, CPAD - c)) for c in range(0, CPAD, 512)]
N_WARMUP = 8                 # dummy matmuls to warm the PE HAM clock gate

LAST_RESULT = None
_CACHE = {}


def _bounds(groups):
    out, c = [], 0
    for g in groups:
        out.append((c, g))
        c += g
    return out


def _build_nc():
    from concourse import bass, bacc, tile, mybir
    from contextlib import ExitStack

    f32 = mybir.dt.float32
    bf16 = mybir.dt.bfloat16

    nc = bacc.Bacc()
    # [p, bt*512 + d*128 + b_local] = (S*x_hat)[bt*128 + b_local, d*128 + p]
    in_e = nc.declare_dram_parameter("insT", [128, BT * DCH * 128], bf16, isOutput=False)
    # group-major: [p, wg_off*DCH + d*wcols + c_local] = w_hat[c, d*128 + p]
    wt_e = nc.declare_dram_parameter("wt", [128, DCH * CPAD], bf16, isOutput=False)
    # group-major: [p, og_off*BT + bt*ocols + c_local] = out[bt*128 + p, c]
    out_e = nc.declare_dram_parameter("out", [128, BT * CPAD], bf16, isOutput=True)

    wbounds = _bounds(W_GROUPS)
    obounds = _bounds(O_GROUPS)

    with tile.TileContext(nc) as tc, ExitStack() as ctx:
        cpool = ctx.enter_context(tc.tile_pool(name="const", bufs=1))
        wpool = ctx.enter_context(tc.tile_pool(name="wts", bufs=3))
        opool = ctx.enter_context(tc.tile_pool(name="outb", bufs=3))
        pm = ctx.enter_context(tc.tile_pool(name="pm", bufs=6, space="PSUM"))
        pw = ctx.enter_context(tc.tile_pool(name="pwarm", bufs=1, space="PSUM"))

        # single input DMA, issued first on the sync ring ahead of weights
        in_sT = cpool.tile([128, BT * DCH * 128], bf16)
        nc.sync.dma_start(in_sT[:], in_e[:, :])

        # Warm the PE clock gate while weight group 0 is still in flight.
        wps = pw.tile([128, 128], f32)
        for _ in range(N_WARMUP):
            nc.tensor.matmul(
                wps[:], in_sT[:, 0:128], in_sT[:, 0:128], start=True, stop=True
            )

        wt_tiles = {}   # wg index -> (tile, wg_off, wcols)
        ob = None
        eng_i = 0
        wg_i = -1
        og_i = -1
        for c0, ncols in CHUNKS:
            # weight group prefetch boundary
            if wg_i + 1 < len(wbounds) and c0 == wbounds[wg_i + 1][0]:
                wg_i += 1
                woff, wcols = wbounds[wg_i]
                wt_t = wpool.tile([128, DCH, wcols], bf16, tag="wt")
                nc.sync.dma_start(
                    wt_t[:], wt_e[:, DCH * woff: DCH * (woff + wcols)]
                )
                wt_tiles = {"t": wt_t, "off": woff, "cols": wcols}
            # output group boundary
            if og_i + 1 < len(obounds) and c0 == obounds[og_i + 1][0]:
                og_i += 1
                ooff, ocols = obounds[og_i]
                ob = opool.tile([128, BT, ocols], bf16, tag="ob")

            wt_t, woff, wcols = wt_tiles["t"], wt_tiles["off"], wt_tiles["cols"]
            jl = c0 - woff
            for bt in range(BT):
                ps = pm.tile([128, ncols], f32, tag="ps")
                for d in range(DCH):
                    nc.tensor.matmul(
                        ps[:],
                        in_sT[:, bt * 512 + d * 128: bt * 512 + (d + 1) * 128],
                        wt_t[:, d, jl:jl + ncols],
                        start=(d == 0),
                        stop=(d == DCH - 1),
                    )
                dst = ob[:, bt, c0 - ooff: c0 - ooff + ncols]
                if eng_i % 2 == 0:
                    nc.vector.tensor_copy(dst, ps[:])
                else:
                    nc.scalar.copy(dst, ps[:])
                eng_i += 1

            # end of output group -> store it (gpsimd SWDGE queue, so stores
            # never queue behind weight prefetches)
            if c0 + ncols == ooff + ocols:
                nc.gpsimd.dma_start(
                    out_e[:, BT * ooff: BT * (ooff + ocols)], ob[:]
                )
    nc.finalize()
    return nc


def _get_nc():
    if "nc" not in _CACHE:
        _CACHE["nc"] = _build_nc()
    return _CACHE["nc"]


def _host_prep(inp, w):
    """Normalize on host; returns (insT bf16 [BT,128,512], per-core wt bf16)."""
    bf16 = ml_dtypes.bfloat16
    xn = inp / np.maximum(np.linalg.norm(inp, axis=1, keepdims=True), 1e-12)
    a = (S * xn).T.astype(bf16)                      # [D, B]
    # [p, bt*512 + d*128 + bl] = a[d*128 + p, bt*128 + bl]
    insT = np.ascontiguousarray(
        a.reshape(DCH, 128, BT, 128).transpose(1, 2, 0, 3).reshape(128, BT * DCH * 128)
    )

    wn = w / np.maximum(np.linalg.norm(w, axis=1, keepdims=True), 1e-12)
    wn = wn.astype(bf16)                             # [C, D]
    wts = []
    for k in range(NCORES):
        blk = wn[k * CSH:(k + 1) * CSH].T            # [D, CSH]
        wk = np.zeros((DCH, 128, CPAD), dtype=bf16)  # [d, p, c]
        wk[:, :, :CSH] = blk.reshape(DCH, 128, CSH)
        # group-major flatten: per partition [wg0: d-major cols | wg1: ...]
        wkp = wk.transpose(1, 0, 2)                  # [p, d, c]
        segs = [
            wkp[:, :, off:off + cols].reshape(128, DCH * cols)
            for off, cols in _bounds(W_GROUPS)
        ]
        wts.append(np.ascontiguousarray(np.concatenate(segs, axis=1)))
    return insT, wts


def _host_margin(inp, lbl, w):
    """Exact (float64) ArcFace margin values S*phi for each row."""
    x = inp.astype(np.float64)
    wl = w[lbl].astype(np.float64)
    xn = x / np.maximum(np.linalg.norm(x, axis=1, keepdims=True), 1e-12)
    wln = wl / np.maximum(np.linalg.norm(wl, axis=1, keepdims=True), 1e-12)
    cosl = np.clip(np.sum(xn * wln, axis=1), -1.0, 1.0)
    sine = np.sqrt(np.clip(1.0 - cosl * cosl, 1e-9, 1.0))
    phi = cosl * COS_M - sine * SIN_M
    phi = np.where(cosl > TH, phi, cosl - MM)
    return (phi * S).astype(np.float32)


def kernel(input, label, weight):
    global LAST_RESULT
    from concourse.bass_utils import run_bass_kernel_spmd

    inp = np.ascontiguousarray(np.asarray(input, dtype=np.float32))
    lbl = np.asarray(label).astype(np.int64)
    w = np.ascontiguousarray(np.asarray(weight, dtype=np.float32))

    insT, wts = _host_prep(inp, w)
    in_maps = [{"insT": insT, "wt": wts[k]} for k in range(NCORES)]

    nc = _get_nc()
    res = run_bass_kernel_spmd(nc, in_maps, core_ids=list(range(NCORES)))
    LAST_RESULT = res
    outs = res.results

    full = np.empty((B, C), dtype=np.float32)
    asm = np.empty((BT, 128, CPAD), dtype=np.float32)
    for k in range(NCORES):
        arr = np.asarray(outs[k]["out"])             # [128, BT*CPAD] group-major
        for off, cols in _bounds(O_GROUPS):
            seg = arr[:, BT * off: BT * (off + cols)].reshape(128, BT, cols)
            asm[:, :, off:off + cols] = seg.transpose(1, 0, 2)
        full[:, k * CSH:(k + 1) * CSH] = asm.reshape(B, CPAD)[:, :CSH]
    full[np.arange(B), lbl] = _host_margin(inp, lbl, w)
    return full


# revision 15
# speedup vs baseline: 1.1518x; 1.1518x over previous
"""ArcFace loss kernel for 8 Trainium2 NeuronCores.

Strategy (class-parallel, Partial-FC style):
  - Host pre-normalizes weight rows (w_hat = w/||w||, bf16) and input rows
    (S*x_hat, bf16, transposed); each core's device program is a pure bf16
    GEMM: out[b, c] = <S*x_hat_b, w_hat_c> for its 12500-class shard
    (padded to 12544), streamed class-group by class-group.
  - Input chunks [128d, 128b] are the stationary operand; weight columns
    stream as the moving operand, so PSUM comes out batch-major [128b, Nc].
  - All DRAM tensors use group-major layouts so every DMA is one contiguous
    run per partition (max descriptor size, minimal descriptor count).
  - Output is written bf16 (halves the dominant DMA-write traffic) and
    upcast to f32 on the host during the gather. Class groups shrink toward
    the end of the sweep so the final output DMAs flush right behind the
    last matmuls.
  - The ArcFace margin touches one element per row; the host computes the
    512 margin values phi(cos(b, label_b)) in float64 and scatters them
    into the gathered [B, C] output.
"""

import math
import os
import sys

import numpy as np

for _p in ("/opt/trn_rl_repo",):
    if os.path.isdir(_p) and _p not in sys.path:
        sys.path.insert(0, _p)

import ml_dtypes

S = 30.0
MARGIN = 0.5
COS_M = math.cos(MARGIN)
SIN_M = math.sin(MARGIN)
TH = math.cos(math.pi - MARGIN)
MM = math.sin(math.pi - MARGIN) * MARGIN

B, D, C = 512, 512, 100000
NCORES = 8
CSH = C // NCORES            # 12500 classes per core
CPAD = 12544                 # padded to 98*128
BT = B // 128                # 4 batch tiles
DCH = D // 128               # 4 contraction chunks

# Weight-prefetch groups (columns). Small first groups so compute starts
# early and the sync ring stays ahead of the matmul sweep.
W_GROUPS = [512, 512, 512, 1024, 1024, 2048, 2048, 2048, 2048, 512, 256]
# Output-store groups (columns). Small tail groups so the final stores
# flush immediately behind the last matmuls.
O_GROUPS = [512, 1024, 2048, 2048, 2048, 2048, 1536, 1024, 256]
assert sum(W_GROUPS) == CPAD and sum(O_GROUPS) == CPAD
# PSUM chunk list (c0, ncols): 512-wide, subdividing both group patterns.
CHUNKS = [(c, min(512, CPAD - c)) for c in range(0, CPAD, 512)]
N_WARMUP = 8                 # dummy matmuls to warm the PE HAM clock gate

LAST_RESULT = None
_CACHE = {}


def _bounds(groups):
    out, c = [], 0
    for g in groups:
        out.append((c, g))
        c += g
    return out


def _build_nc():
    from concourse import bass, bacc, tile, mybir
    from contextlib import ExitStack

    f32 = mybir.dt.float32
    bf16 = mybir.dt.bfloat16

    nc = bacc.Bacc()
    # [p, bt*512 + d*128 + b_local] = (S*x_hat)[bt*128 + b_local, d*128 + p]
    in_e = nc.declare_dram_parameter("insT", [128, BT * DCH * 128], bf16, isOutput=False)
    # [d, p, c] = w_hat[c, d*128 + p]
    wt_e = nc.declare_dram_parameter("wt", [DCH, 128, CPAD], bf16, isOutput=False)
    # [bt, p, c] = out[bt*128 + p, c]
    out_e = nc.declare_dram_parameter("out", [BT, 128, CPAD], bf16, isOutput=True)

    wbounds = _bounds(W_GROUPS)
    obounds = _bounds(O_GROUPS)

    with tile.TileContext(nc) as tc, ExitStack() as ctx:
        cpool = ctx.enter_context(tc.tile_pool(name="const", bufs=1))
        wpool = ctx.enter_context(tc.tile_pool(name="wts", bufs=3))
        opool = ctx.enter_context(tc.tile_pool(name="outb", bufs=3))
        pm = ctx.enter_context(tc.tile_pool(name="pm", bufs=6, space="PSUM"))
        pw = ctx.enter_context(tc.tile_pool(name="pwarm", bufs=1, space="PSUM"))

        # single input DMA, issued first on the sync ring ahead of weights
        in_sT = cpool.tile([128, BT * DCH * 128], bf16)
        nc.sync.dma_start(in_sT[:], in_e[:, :])

        # Warm the PE clock gate while weight group 0 is still in flight.
        wps = pw.tile([128, 128], f32)
        for _ in range(N_WARMUP):
            nc.tensor.matmul(
                wps[:], in_sT[:, 0:128], in_sT[:, 0:128], start=True, stop=True
            )

        wt_tiles = {}   # wg index -> (tile, wg_off, wcols)
        ob = None
        eng_i = 0
        wg_i = -1
        og_i = -1
        for c0, ncols in CHUNKS:
            # weight group prefetch boundary
            if wg_i + 1 < len(wbounds) and c0 == wbounds[wg_i + 1][0]:
                wg_i += 1
                woff, wcols = wbounds[wg_i]
                wt_t = wpool.tile([128, DCH, wcols], bf16, tag="wt")
                nc.sync.dma_start(
                    wt_t[:],
                    wt_e[:, :, woff:woff + wcols].rearrange("d p c -> p d c"),
                )
                wt_tiles = {"t": wt_t, "off": woff, "cols": wcols}
            # output group boundary
            if og_i + 1 < len(obounds) and c0 == obounds[og_i + 1][0]:
                og_i += 1
                ooff, ocols = obounds[og_i]
                ob = opool.tile([128, BT, ocols], bf16, tag="ob")

            wt_t, woff, wcols = wt_tiles["t"], wt_tiles["off"], wt_tiles["cols"]
            jl = c0 - woff
            for bt in range(BT):
                ps = pm.tile([128, ncols], f32, tag="ps")
                for d in range(DCH):
                    nc.tensor.matmul(
                        ps[:],
                        in_sT[:, bt * 512 + d * 128: bt * 512 + (d + 1) * 128],
                        wt_t[:, d, jl:jl + ncols],
                        start=(d == 0),
                        stop=(d == DCH - 1),
                    )
                dst = ob[:, bt, c0 - ooff: c0 - ooff + ncols]
                if eng_i % 2 == 0:
                    nc.vector.tensor_copy(dst, ps[:])
                else:
                    nc.scalar.copy(dst, ps[:])
                eng_i += 1

            # end of output group -> store it (gpsimd SWDGE queue, so stores
            # never queue behind weight prefetches)
            if c0 + ncols == ooff + ocols:
                nc.gpsimd.dma_start(
                    out_e[:, :, ooff:ooff + ocols].rearrange("t p c -> p t c"),
                    ob[:],
                )
    nc.finalize()
    return nc


def _get_nc():
    if "nc" not in _CACHE:
        _CACHE["nc"] = _build_nc()
    return _CACHE["nc"]


def _host_prep(inp, w):
    """Normalize on host; returns (insT bf16 [BT,128,512], per-core wt bf16)."""
    bf16 = ml_dtypes.bfloat16
    xn = inp / np.maximum(np.linalg.norm(inp, axis=1, keepdims=True), 1e-12)
    a = (S * xn).T.astype(bf16)                      # [D, B]
    # [p, bt*512 + d*128 + bl] = a[d*128 + p, bt*128 + bl]
    insT = np.ascontiguousarray(
        a.reshape(DCH, 128, BT, 128).transpose(1, 2, 0, 3).reshape(128, BT * DCH * 128)
    )

    wn = w / np.maximum(np.linalg.norm(w, axis=1, keepdims=True), 1e-12)
    wn = wn.astype(bf16)                             # [C, D]
    wts = []
    for k in range(NCORES):
        blk = wn[k * CSH:(k + 1) * CSH].T            # [D, CSH]
        wk = np.zeros((DCH, 128, CPAD), dtype=bf16)  # [d, p, c]
        wk[:, :, :CSH] = blk.reshape(DCH, 128, CSH)
        wts.append(wk)
    return insT, wts


def _host_margin(inp, lbl, w):
    """Exact (float64) ArcFace margin values S*phi for each row."""
    x = inp.astype(np.float64)
    wl = w[lbl].astype(np.float64)
    xn = x / np.maximum(np.linalg.norm(x, axis=1, keepdims=True), 1e-12)
    wln = wl / np.maximum(np.linalg.norm(wl, axis=1, keepdims=True), 1e-12)
    cosl = np.clip(np.sum(xn * wln, axis=1), -1.0, 1.0)
    sine = np.sqrt(np.clip(1.0 - cosl * cosl, 1e-9, 1.0))
    phi = cosl * COS_M - sine * SIN_M
    phi = np.where(cosl > TH, phi, cosl - MM)
    return (phi * S).astype(np.float32)


def kernel(input, label, weight):
    global LAST_RESULT
    from concourse.bass_utils import run_bass_kernel_spmd

    inp = np.ascontiguousarray(np.asarray(input, dtype=np.float32))
    lbl = np.asarray(label).astype(np.int64)
    w = np.ascontiguousarray(np.asarray(weight, dtype=np.float32))

    insT, wts = _host_prep(inp, w)
    in_maps = [{"insT": insT, "wt": wts[k]} for k in range(NCORES)]

    nc = _get_nc()
    res = run_bass_kernel_spmd(nc, in_maps, core_ids=list(range(NCORES)))
    LAST_RESULT = res
    outs = res.results

    full = np.empty((B, C), dtype=np.float32)
    for k in range(NCORES):
        arr = np.asarray(outs[k]["out"])             # [BT, 128, CPAD]
        full[:, k * CSH:(k + 1) * CSH] = arr.reshape(B, CPAD)[:, :CSH].astype(
            np.float32
        )
    full[np.arange(B), lbl] = _host_margin(inp, lbl, w)
    return full


# revision 18
# speedup vs baseline: 1.1890x; 1.0324x over previous
"""ArcFace loss kernel for 8 Trainium2 NeuronCores.

Strategy (class-parallel, Partial-FC style):
  - Host pre-normalizes weight rows (w_hat = w/||w||, bf16) and input rows
    (S*x_hat, bf16, transposed); each core's device program is a pure bf16
    GEMM: out[b, c] = <S*x_hat_b, w_hat_c> for its 12500-class shard
    (padded to 12544), streamed class-group by class-group.
  - Input chunks [128d, 128b] are the stationary operand; weight columns
    stream as the moving operand, so PSUM comes out batch-major [128b, Nc].
  - All DRAM tensors use group-major layouts so every DMA is one contiguous
    run per partition (max descriptor size, minimal descriptor count).
  - Output is written bf16 (halves the dominant DMA-write traffic) and
    upcast to f32 on the host during the gather. Class groups shrink toward
    the end of the sweep so the final output DMAs flush right behind the
    last matmuls.
  - The ArcFace margin touches one element per row; the host computes the
    512 margin values phi(cos(b, label_b)) in float64 and scatters them
    into the gathered [B, C] output.
"""

import math
import os
import sys

import numpy as np

for _p in ("/opt/trn_rl_repo",):
    if os.path.isdir(_p) and _p not in sys.path:
        sys.path.insert(0, _p)

import ml_dtypes

S = 30.0
MARGIN = 0.5
COS_M = math.cos(MARGIN)
SIN_M = math.sin(MARGIN)
TH = math.cos(math.pi - MARGIN)
MM = math.sin(math.pi - MARGIN) * MARGIN

B, D, C = 512, 512, 100000
NCORES = 8
CSH = C // NCORES            # 12500 classes per core
CPAD = 12544                 # padded to 98*128
BT = B // 128                # 4 batch tiles
DCH = D // 128               # 4 contraction chunks

# Weight-prefetch groups (columns). Small first groups so compute starts
# early and the sync ring stays ahead of the matmul sweep.
W_GROUPS = [512, 512, 512, 1024, 1024, 2048, 2048, 2048, 2048, 512, 256]
# Output-store groups (columns). Small tail groups so the final stores
# flush immediately behind the last matmuls.
O_GROUPS = [512, 1024, 2048, 2048, 2048, 2048, 1536, 512, 512, 256]
assert sum(W_GROUPS) == CPAD and sum(O_GROUPS) == CPAD
# PSUM chunk list (c0, ncols): 512-wide, subdividing both group patterns.
CHUNKS = [(c, min(512, CPAD - c)) for c in range(0, CPAD, 512)]
N_WARMUP = 36                # dummy matmuls to warm the PE HAM clock gate

LAST_RESULT = None
_CACHE = {}


def _bounds(groups):
    out, c = [], 0
    for g in groups:
        out.append((c, g))
        c += g
    return out


def _build_nc():
    from concourse import bass, bacc, tile, mybir
    from contextlib import ExitStack

    f32 = mybir.dt.float32
    bf16 = mybir.dt.bfloat16

    nc = bacc.Bacc()
    # [p, bt*512 + d*128 + b_local] = (S*x_hat)[bt*128 + b_local, d*128 + p]
    in_e = nc.declare_dram_parameter("insT", [128, BT * DCH * 128], bf16, isOutput=False)
    # [d, p, c] = w_hat[c, d*128 + p]
    wt_e = nc.declare_dram_parameter("wt", [DCH, 128, CPAD], bf16, isOutput=False)
    # [bt, p, c] = out[bt*128 + p, c]
    out_e = nc.declare_dram_parameter("out", [BT, 128, CPAD], bf16, isOutput=True)

    wbounds = _bounds(W_GROUPS)
    obounds = _bounds(O_GROUPS)

    with tile.TileContext(nc) as tc, ExitStack() as ctx:
        cpool = ctx.enter_context(tc.tile_pool(name="const", bufs=1))
        wpool = ctx.enter_context(tc.tile_pool(name="wts", bufs=3))
        opool = ctx.enter_context(tc.tile_pool(name="outb", bufs=3))
        pm = ctx.enter_context(tc.tile_pool(name="pm", bufs=6, space="PSUM"))
        pw = ctx.enter_context(tc.tile_pool(name="pwarm", bufs=1, space="PSUM"))

        # weight group 0 first on the sync ring, then the input DMA
        wt0 = wpool.tile([128, DCH, W_GROUPS[0]], bf16, tag="wt")
        nc.sync.dma_start(
            wt0[:],
            wt_e[:, :, 0:W_GROUPS[0]].rearrange("d p c -> p d c"),
        )
        in_sT = cpool.tile([128, BT * DCH * 128], bf16)
        nc.sync.dma_start(in_sT[:], in_e[:, :])

        # Warm the PE clock gate on a memset tile — no DMA dependency, so
        # the PE is busy from right after the preamble and is at full clock
        # by the time real operands land.
        wtile = cpool.tile([128, 128], bf16)
        nc.vector.memset(wtile[:], 0.5)
        wps = pw.tile([128, 128], f32)
        for _ in range(N_WARMUP):
            nc.tensor.matmul(
                wps[:], wtile[:], wtile[:], start=True, stop=True
            )

        wt_tiles = {}   # wg index -> (tile, wg_off, wcols)
        ob = None
        eng_i = 0
        wg_i = -1
        og_i = -1
        for c0, ncols in CHUNKS:
            # weight group prefetch boundary
            if wg_i + 1 < len(wbounds) and c0 == wbounds[wg_i + 1][0]:
                wg_i += 1
                woff, wcols = wbounds[wg_i]
                if wg_i == 0:
                    wt_t = wt0
                else:
                    wt_t = wpool.tile([128, DCH, wcols], bf16, tag="wt")
                    nc.sync.dma_start(
                        wt_t[:],
                        wt_e[:, :, woff:woff + wcols].rearrange("d p c -> p d c"),
                    )
                wt_tiles = {"t": wt_t, "off": woff, "cols": wcols}
            # output group boundary
            if og_i + 1 < len(obounds) and c0 == obounds[og_i + 1][0]:
                og_i += 1
                ooff, ocols = obounds[og_i]
                ob = opool.tile([128, BT, ocols], bf16, tag="ob")

            wt_t, woff, wcols = wt_tiles["t"], wt_tiles["off"], wt_tiles["cols"]
            jl = c0 - woff
            for bt in range(BT):
                ps = pm.tile([128, ncols], f32, tag="ps")
                for d in range(DCH):
                    nc.tensor.matmul(
                        ps[:],
                        in_sT[:, bt * 512 + d * 128: bt * 512 + (d + 1) * 128],
                        wt_t[:, d, jl:jl + ncols],
                        start=(d == 0),
                        stop=(d == DCH - 1),
                    )
                dst = ob[:, bt, c0 - ooff: c0 - ooff + ncols]
                if eng_i % 2 == 0:
                    nc.vector.tensor_copy(dst, ps[:])
                else:
                    nc.scalar.copy(dst, ps[:])
                eng_i += 1

            # end of output group -> store it (gpsimd SWDGE queue, so stores
            # never queue behind weight prefetches)
            if c0 + ncols == ooff + ocols:
                nc.gpsimd.dma_start(
                    out_e[:, :, ooff:ooff + ocols].rearrange("t p c -> p t c"),
                    ob[:],
                )
    nc.finalize()
    return nc


def _get_nc():
    if "nc" not in _CACHE:
        _CACHE["nc"] = _build_nc()
    return _CACHE["nc"]


def _host_prep(inp, w):
    """Normalize on host; returns (insT bf16 [BT,128,512], per-core wt bf16)."""
    bf16 = ml_dtypes.bfloat16
    xn = inp / np.maximum(np.linalg.norm(inp, axis=1, keepdims=True), 1e-12)
    a = (S * xn).T.astype(bf16)                      # [D, B]
    # [p, bt*512 + d*128 + bl] = a[d*128 + p, bt*128 + bl]
    insT = np.ascontiguousarray(
        a.reshape(DCH, 128, BT, 128).transpose(1, 2, 0, 3).reshape(128, BT * DCH * 128)
    )

    wn = w / np.maximum(np.linalg.norm(w, axis=1, keepdims=True), 1e-12)
    wn = wn.astype(bf16)                             # [C, D]
    wts = []
    for k in range(NCORES):
        blk = wn[k * CSH:(k + 1) * CSH].T            # [D, CSH]
        wk = np.zeros((DCH, 128, CPAD), dtype=bf16)  # [d, p, c]
        wk[:, :, :CSH] = blk.reshape(DCH, 128, CSH)
        wts.append(wk)
    return insT, wts


def _host_margin(inp, lbl, w):
    """Exact (float64) ArcFace margin values S*phi for each row."""
    x = inp.astype(np.float64)
    wl = w[lbl].astype(np.float64)
    xn = x / np.maximum(np.linalg.norm(x, axis=1, keepdims=True), 1e-12)
    wln = wl / np.maximum(np.linalg.norm(wl, axis=1, keepdims=True), 1e-12)
    cosl = np.clip(np.sum(xn * wln, axis=1), -1.0, 1.0)
    sine = np.sqrt(np.clip(1.0 - cosl * cosl, 1e-9, 1.0))
    phi = cosl * COS_M - sine * SIN_M
    phi = np.where(cosl > TH, phi, cosl - MM)
    return (phi * S).astype(np.float32)


def kernel(input, label, weight):
    global LAST_RESULT
    from concourse.bass_utils import run_bass_kernel_spmd

    inp = np.ascontiguousarray(np.asarray(input, dtype=np.float32))
    lbl = np.asarray(label).astype(np.int64)
    w = np.ascontiguousarray(np.asarray(weight, dtype=np.float32))

    insT, wts = _host_prep(inp, w)
    in_maps = [{"insT": insT, "wt": wts[k]} for k in range(NCORES)]

    nc = _get_nc()
    res = run_bass_kernel_spmd(nc, in_maps, core_ids=list(range(NCORES)))
    LAST_RESULT = res
    outs = res.results

    full = np.empty((B, C), dtype=np.float32)
    for k in range(NCORES):
        arr = np.asarray(outs[k]["out"])             # [BT, 128, CPAD]
        full[:, k * CSH:(k + 1) * CSH] = arr.reshape(B, CPAD)[:, :CSH].astype(
            np.float32
        )
    full[np.arange(B), lbl] = _host_margin(inp, lbl, w)
    return full


# revision 21
# speedup vs baseline: 1.2152x; 1.0220x over previous
"""ArcFace loss kernel for 8 Trainium2 NeuronCores.

Strategy (class-parallel, Partial-FC style):
  - Host pre-normalizes weight rows (w_hat = w/||w||, bf16) and input rows
    (S*x_hat, bf16, transposed); each core's device program is a pure bf16
    GEMM: out[b, c] = <S*x_hat_b, w_hat_c> for its 12500-class shard
    (padded to 12544), streamed class-group by class-group.
  - Input chunks [128d, 128b] are the stationary operand; weight columns
    stream as the moving operand, so PSUM comes out batch-major [128b, Nc].
  - All DRAM tensors use group-major layouts so every DMA is one contiguous
    run per partition (max descriptor size, minimal descriptor count).
  - Output is written bf16 (halves the dominant DMA-write traffic) and
    upcast to f32 on the host during the gather. Class groups shrink toward
    the end of the sweep so the final output DMAs flush right behind the
    last matmuls.
  - The ArcFace margin touches one element per row; the host computes the
    512 margin values phi(cos(b, label_b)) in float64 and scatters them
    into the gathered [B, C] output.
"""

import math
import os
import sys

import numpy as np

for _p in ("/opt/trn_rl_repo",):
    if os.path.isdir(_p) and _p not in sys.path:
        sys.path.insert(0, _p)

import ml_dtypes

S = 30.0
MARGIN = 0.5
COS_M = math.cos(MARGIN)
SIN_M = math.sin(MARGIN)
TH = math.cos(math.pi - MARGIN)
MM = math.sin(math.pi - MARGIN) * MARGIN

B, D, C = 512, 512, 100000
NCORES = 8
CSH = C // NCORES            # 12500 classes per core
CPAD = 12544                 # padded to 98*128
BT = B // 128                # 4 batch tiles
DCH = D // 128               # 4 contraction chunks

# Weight-prefetch groups (columns). Small first groups so compute starts
# early and the sync ring stays ahead of the matmul sweep.
W_GROUPS = [512, 512, 512, 1024, 1024, 2048, 2048, 2048, 2048, 512, 256]
# Output-store groups (columns). Small tail groups so the final stores
# flush immediately behind the last matmuls.
O_GROUPS = [512, 1024, 2048, 2048, 2048, 2048, 1536, 512, 512, 256]
assert sum(W_GROUPS) == CPAD and sum(O_GROUPS) == CPAD
# PSUM chunk list (c0, ncols): 512-wide, subdividing both group patterns.
CHUNKS = [(c, min(512, CPAD - c)) for c in range(0, CPAD, 512)]
N_WARMUP = 36                # dummy matmuls to warm the PE HAM clock gate

LAST_RESULT = None
_CACHE = {}


def _bounds(groups):
    out, c = [], 0
    for g in groups:
        out.append((c, g))
        c += g
    return out


def _build_nc():
    from concourse import bass, bacc, tile, mybir
    from contextlib import ExitStack

    f32 = mybir.dt.float32
    bf16 = mybir.dt.bfloat16

    nc = bacc.Bacc()
    # [p, bt*512 + d*128 + b_local] = (S*x_hat)[bt*128 + b_local, d*128 + p]
    in_e = nc.declare_dram_parameter("insT", [128, BT * DCH * 128], bf16, isOutput=False)
    # [d, p, c] = w_hat[c, d*128 + p]
    wt_e = nc.declare_dram_parameter("wt", [DCH, 128, CPAD], bf16, isOutput=False)
    # group-major: [p, og_off*BT + bt*ocols + c_local] = out[bt*128 + p, c]
    out_e = nc.declare_dram_parameter("out", [128, BT * CPAD], bf16, isOutput=True)

    wbounds = _bounds(W_GROUPS)
    obounds = _bounds(O_GROUPS)

    with tile.TileContext(nc) as tc, ExitStack() as ctx:
        cpool = ctx.enter_context(tc.tile_pool(name="const", bufs=1))
        wpool = ctx.enter_context(tc.tile_pool(name="wts", bufs=3))
        opool = ctx.enter_context(tc.tile_pool(name="outb", bufs=3))
        pm = ctx.enter_context(tc.tile_pool(name="pm", bufs=6, space="PSUM"))
        pw = ctx.enter_context(tc.tile_pool(name="pwarm", bufs=1, space="PSUM"))

        # weight group 0 first on the sync ring, then the input DMA
        wt0 = wpool.tile([128, DCH, W_GROUPS[0]], bf16, tag="wt")
        nc.sync.dma_start(
            wt0[:],
            wt_e[:, :, 0:W_GROUPS[0]].rearrange("d p c -> p d c"),
        )
        in_sT = cpool.tile([128, BT * DCH * 128], bf16)
        nc.sync.dma_start(in_sT[:], in_e[:, :])

        # Warm the PE clock gate on a memset tile — no DMA dependency, so
        # the PE is busy from right after the preamble and is at full clock
        # by the time real operands land.
        wtile = cpool.tile([128, 128], bf16)
        nc.vector.memset(wtile[:], 0.5)
        wps = pw.tile([128, 128], f32)
        for _ in range(N_WARMUP):
            nc.tensor.matmul(
                wps[:], wtile[:], wtile[:], start=True, stop=True
            )

        wt_tiles = {}   # wg index -> (tile, wg_off, wcols)
        ob = None
        eng_i = 0
        wg_i = -1
        og_i = -1
        for c0, ncols in CHUNKS:
            # weight group prefetch boundary
            if wg_i + 1 < len(wbounds) and c0 == wbounds[wg_i + 1][0]:
                wg_i += 1
                woff, wcols = wbounds[wg_i]
                if wg_i == 0:
                    wt_t = wt0
                else:
                    wt_t = wpool.tile([128, DCH, wcols], bf16, tag="wt")
                    nc.sync.dma_start(
                        wt_t[:],
                        wt_e[:, :, woff:woff + wcols].rearrange("d p c -> p d c"),
                    )
                wt_tiles = {"t": wt_t, "off": woff, "cols": wcols}
            # output group boundary
            if og_i + 1 < len(obounds) and c0 == obounds[og_i + 1][0]:
                og_i += 1
                ooff, ocols = obounds[og_i]
                ob = opool.tile([128, BT, ocols], bf16, tag="ob")

            wt_t, woff, wcols = wt_tiles["t"], wt_tiles["off"], wt_tiles["cols"]
            jl = c0 - woff
            for bt in range(BT):
                ps = pm.tile([128, ncols], f32, tag="ps")
                for d in range(DCH):
                    nc.tensor.matmul(
                        ps[:],
                        in_sT[:, bt * 512 + d * 128: bt * 512 + (d + 1) * 128],
                        wt_t[:, d, jl:jl + ncols],
                        start=(d == 0),
                        stop=(d == DCH - 1),
                    )
                dst = ob[:, bt, c0 - ooff: c0 - ooff + ncols]
                if eng_i % 2 == 0:
                    nc.vector.tensor_copy(dst, ps[:])
                else:
                    nc.scalar.copy(dst, ps[:])
                eng_i += 1

            # end of output group -> store it (gpsimd SWDGE queue, so stores
            # never queue behind weight prefetches)
            if c0 + ncols == ooff + ocols:
                nc.gpsimd.dma_start(
                    out_e[:, BT * ooff: BT * (ooff + ocols)], ob[:]
                )
    nc.finalize()
    return nc


def _get_nc():
    if "nc" not in _CACHE:
        _CACHE["nc"] = _build_nc()
    return _CACHE["nc"]


def _host_prep(inp, w):
    """Normalize on host; returns (insT bf16 [BT,128,512], per-core wt bf16)."""
    bf16 = ml_dtypes.bfloat16
    xn = inp / np.maximum(np.linalg.norm(inp, axis=1, keepdims=True), 1e-12)
    a = (S * xn).T.astype(bf16)                      # [D, B]
    # [p, bt*512 + d*128 + bl] = a[d*128 + p, bt*128 + bl]
    insT = np.ascontiguousarray(
        a.reshape(DCH, 128, BT, 128).transpose(1, 2, 0, 3).reshape(128, BT * DCH * 128)
    )

    wn = w / np.maximum(np.linalg.norm(w, axis=1, keepdims=True), 1e-12)
    wn = wn.astype(bf16)                             # [C, D]
    wts = []
    for k in range(NCORES):
        blk = wn[k * CSH:(k + 1) * CSH].T            # [D, CSH]
        wk = np.zeros((DCH, 128, CPAD), dtype=bf16)  # [d, p, c]
        wk[:, :, :CSH] = blk.reshape(DCH, 128, CSH)
        wts.append(wk)
    return insT, wts


def _host_margin(inp, lbl, w):
    """Exact (float64) ArcFace margin values S*phi for each row."""
    x = inp.astype(np.float64)
    wl = w[lbl].astype(np.float64)
    xn = x / np.maximum(np.linalg.norm(x, axis=1, keepdims=True), 1e-12)
    wln = wl / np.maximum(np.linalg.norm(wl, axis=1, keepdims=True), 1e-12)
    cosl = np.clip(np.sum(xn * wln, axis=1), -1.0, 1.0)
    sine = np.sqrt(np.clip(1.0 - cosl * cosl, 1e-9, 1.0))
    phi = cosl * COS_M - sine * SIN_M
    phi = np.where(cosl > TH, phi, cosl - MM)
    return (phi * S).astype(np.float32)


def kernel(input, label, weight):
    global LAST_RESULT
    from concourse.bass_utils import run_bass_kernel_spmd

    inp = np.ascontiguousarray(np.asarray(input, dtype=np.float32))
    lbl = np.asarray(label).astype(np.int64)
    w = np.ascontiguousarray(np.asarray(weight, dtype=np.float32))

    insT, wts = _host_prep(inp, w)
    in_maps = [{"insT": insT, "wt": wts[k]} for k in range(NCORES)]

    nc = _get_nc()
    res = run_bass_kernel_spmd(nc, in_maps, core_ids=list(range(NCORES)))
    LAST_RESULT = res
    outs = res.results

    full = np.empty((B, C), dtype=np.float32)
    asm = np.empty((BT, 128, CPAD), dtype=np.float32)
    for k in range(NCORES):
        arr = np.asarray(outs[k]["out"])             # [128, BT*CPAD] group-major
        for off, cols in _bounds(O_GROUPS):
            seg = arr[:, BT * off: BT * (off + cols)].reshape(128, BT, cols)
            asm[:, :, off:off + cols] = seg.transpose(1, 0, 2)
        full[:, k * CSH:(k + 1) * CSH] = asm.reshape(B, CPAD)[:, :CSH]
    full[np.arange(B), lbl] = _host_margin(inp, lbl, w)
    return full
